# revision 4
# baseline (speedup 1.0000x reference)
"""Trainium2 Bass kernel for nn_BEATsLayer (BEATs transformer encoder layer).

Strategy: data-parallel over batch. B=8 batch elements -> 8 NeuronCores, one
batch element per core, no collectives. Per core:

  - QKV projections emit qT/kT in transposed [E', T] layout (lhsT=W, rhs=xT)
    and v in natural [T, E'] layout (lhsT=xT, rhs=W), all bf16 with fp32 PSUM.
  - Gated-relative-position gates via a host-folded block-diagonal matmul,
    sigmoid on ScalarE, gate algebra on VectorE.
  - Scores per head: K=64 matmuls, two heads packed in one 128-partition tile
    and run concurrently via tile_position row tiling.
  - logits/32 = (pos_bias * gate/32) + scores  in ONE fused
    scalar_tensor_tensor (per-partition gate scalar); probs = Exp(32*logits)
    on ScalarE with accum_out giving the softmax denominator Z.
  - probs (bf16) DMA-transposed [128,1024] -> [128,8,128]; ctx = pT.T @ v with
    fp32 PSUM accumulation; evac scaled by 1/Z (per-partition).
  - ctx pairs DMA-transposed into ctxT; out-proj + residual (alpha*states+bo
    folded on host); LN via bn_stats/bn_aggr + DVE pow(var+eps, -0.5).
  - FFN: fc1 -> Gelu(+b1 per-partition) -> fc2 -> residual -> LN2 -> out.

All weight casts/transposes/foldings happen on the host inside kernel().
"""

import numpy as np
import ml_dtypes

import concourse.bass as bass
import concourse.mybir as mybir
import concourse.tile as tile
from concourse import bacc
from concourse.bass_utils import run_bass_kernel_spmd

# ---- problem constants (hardcoded per harness contract) ----
T, B, E, H, D, F = 1024, 8, 768, 12, 64, 3072
P = 128
NT = T // P          # 8 t tiles
EC = E // P          # 6 e chunks
FC = F // P          # 24 f chunks
NHP = H // 2         # 6 head pairs
EPS = 1e-5
ALPHA = 24.0 ** 0.25
SCALE_Q = 0.125 / 32.0   # = 1/256 exactly

BF16 = mybir.dt.bfloat16
F32 = mybir.dt.float32
OP = mybir.AluOpType
AF = mybir.ActivationFunctionType

_CACHE = {}


def _build_nc(flags):
    """Build + compile the per-core Bass program. flags: tuple of bools
    (has_bq, has_bk, has_bv, has_bf2, has_b2, has_ln1, has_ln2)."""
    has_bq, has_bk, has_bv, has_bf2, has_b2, has_ln1, has_ln2 = flags

    nc = bacc.Bacc("TRN2", target_bir_lowering=False, debug=False, num_devices=8)

    # ---- DRAM I/O ----
    d_statesT = nc.dram_tensor("statesT", [E, T], BF16, kind="ExternalInput")
    d_spre = nc.dram_tensor("spre", [T, E], F32, kind="ExternalInput")
    d_pos = nc.dram_tensor("pos", [H, T, T], BF16, kind="ExternalInput")
    d_Wq = nc.dram_tensor("Wq", [E, E], BF16, kind="ExternalInput")
    d_Wk = nc.dram_tensor("Wk", [E, E], BF16, kind="ExternalInput")
    d_Wv = nc.dram_tensor("Wv", [E, E], BF16, kind="ExternalInput")
    d_Wo = nc.dram_tensor("Wo", [E, E], BF16, kind="ExternalInput")
    d_W1 = nc.dram_tensor("W1", [E, F], BF16, kind="ExternalInput")
    d_W2 = nc.dram_tensor("W2", [F, E], BF16, kind="ExternalInput")
    d_BD = nc.dram_tensor("BD", [E, 2 * H], BF16, kind="ExternalInput")
    d_A32 = nc.dram_tensor("A32", [H], F32, kind="ExternalInput")
    d_b1 = nc.dram_tensor("b1", [F], F32, kind="ExternalInput")
    d_bq = nc.dram_tensor("bqs", [E], F32, kind="ExternalInput") if has_bq else None
    d_bk = nc.dram_tensor("bk", [E], F32, kind="ExternalInput") if has_bk else None
    d_bv = nc.dram_tensor("bv", [1, E], BF16, kind="ExternalInput") if has_bv else None
    d_bf2 = nc.dram_tensor("bf2", [2 * H], F32, kind="ExternalInput") if has_bf2 else None
    d_b2 = nc.dram_tensor("b2", [1, E], BF16, kind="ExternalInput") if has_b2 else None
    d_ln1 = nc.dram_tensor("ln1gb", [2 * E], F32, kind="ExternalInput") if has_ln1 else None
    d_ln2 = nc.dram_tensor("ln2gb", [2 * E], F32, kind="ExternalInput") if has_ln2 else None
    d_out = nc.dram_tensor("out", [T, E], F32, kind="ExternalOutput")

    def bcast_row(pool, dram_t, n, name):
        t = pool.tile([P, n], F32, name=name)
        a = dram_t.ap()
        bc = bass.AP(tensor=a.tensor, offset=a.offset, ap=[[0, P], a.ap[0]])
        nc.gpsimd.dma_start(out=t[:], in_=bc)
        return t

    with tile.TileContext(nc) as tc:
        with (
            tc.tile_pool(name="psA", bufs=4, space="PSUM") as psA,     # 4 banks
            tc.tile_pool(name="psB", bufs=2, space="PSUM") as psB,     # 2 banks
            tc.tile_pool(name="psC", bufs=2, space="PSUM") as psC,     # 2 banks
            tc.tile_pool(name="persist", bufs=1) as pers,
            tc.tile_pool(name="rot", bufs=3) as rot,
            tc.tile_pool(name="rot2", bufs=2) as rot2,
        ):
            # ---------------- persistent (whole-kernel) tiles ----------------
            xN = pers.tile([P, NT, E], BF16)       # LN1 output, natural
            eps_t = pers.tile([P, 1], F32)
            nc.vector.memset(eps_t[:], EPS)
            xT = pers.tile([P, EC, T], BF16)
            g32 = pers.tile([P, NT, H], F32)       # gate_a_1 / 32 per (t, head)
            b1_t = pers.tile([P, FC], F32)
            A32_t = bcast_row(pers, d_A32, H, "A32_t")
            nc.sync.dma_start(b1_t[:], d_b1.ap().rearrange("(c p) -> p c", p=P))
            bf2_t = bcast_row(pers, d_bf2, 2 * H, "bf2_t") if has_bf2 else None
            ln1_t = bcast_row(pers, d_ln1, 2 * E, "ln1_t") if has_ln1 else None
            ln2_t = bcast_row(pers, d_ln2, 2 * E, "ln2_t") if has_ln2 else None
            ones1 = None
            if has_bv or has_b2:
                ones1 = pers.tile([1, P], BF16)
                nc.vector.memset(ones1[:], 1.0)
            bv_t = b2_t = None
            if has_bv:
                bv_t = pers.tile([1, E], BF16)
                nc.sync.dma_start(bv_t[:], d_bv.ap())
            if has_b2:
                b2_t = pers.tile([1, E], BF16)
                nc.sync.dma_start(b2_t[:], d_b2.ap())

            def _layer_norm(src, dst, ln_t):
                """dst = LN(src) (+affine).  src/dst: [P, E] APs."""
                stats = rot.tile([P, 3, 6], F32, tag="lnst", name="lnst")
                sr = src.rearrange("p (n s) -> p n s", s=256)
                for i in range(3):
                    nc.vector.bn_stats(out=stats[:, i, :], in_=sr[:, i, :])
                mv = rot.tile([P, 2], F32, tag="lnmv", name="lnmv")
                nc.vector.bn_aggr(out=mv[:], in_=stats[:])
                rstd = rot.tile([P, 1], F32, tag="lnrs", name="lnrs")
                nc.scalar.activation(out=rstd[:], in_=mv[:, 1:2], func=AF.Sqrt,
                                     bias=eps_t[:], scale=1.0)
                nc.vector.reciprocal(out=rstd[:], in_=rstd[:])
                if ln_t is None:
                    nc.vector.tensor_scalar(out=dst, in0=src, scalar1=mv[:, 0:1],
                                            scalar2=rstd[:], op0=OP.subtract,
                                            op1=OP.mult)
                else:
                    xn = rot.tile([P, E], F32, tag="lnxn", name="lnxn")
                    nc.vector.tensor_scalar(out=xn[:], in0=src, scalar1=mv[:, 0:1],
                                            scalar2=rstd[:], op0=OP.subtract,
                                            op1=OP.mult)
                    nc.vector.tensor_mul(out=xn[:], in0=xn[:], in1=ln_t[:, 0:E])
                    nc.vector.tensor_add(out=dst, in0=xn[:], in1=ln_t[:, E:2 * E])

            # ================= attention macro-phase =================
            with tc.tile_pool(name="attnP", bufs=1) as atp:
                qT = atp.tile([P, EC, T], BF16)       # q^T * SCALE_Q
                kT = atp.tile([P, EC, T], BF16)
                vN = atp.tile([P, NT, E], BF16)       # v natural [schunk][sloc, e']
                ctxT = atp.tile([P, EC, T], BF16)
                Wo_s = atp.tile([P, EC, E], BF16)
                nc.sync.dma_start(Wo_s[:], d_Wo.ap().rearrange("(c p) e -> p c e", p=P))
                bq_t = bk_t = None
                if has_bq:
                    bq_t = atp.tile([P, EC], F32)
                    nc.sync.dma_start(bq_t[:], d_bq.ap().rearrange("(c p) -> p c", p=P))
                if has_bk:
                    bk_t = atp.tile([P, EC], F32)
                    nc.sync.dma_start(bk_t[:], d_bk.ap().rearrange("(c p) -> p c", p=P))

                # ---------------- phase 1: QKV + gates ----------------
                with tc.tile_pool(name="ph1", bufs=1) as ph1:
                    sT = ph1.tile([P, EC, T], BF16)
                    Wq_s = ph1.tile([P, EC, E], BF16)
                    Wk_s = ph1.tile([P, EC, E], BF16)
                    Wv_s = ph1.tile([P, EC, E], BF16)
                    BD_s = ph1.tile([P, EC, 2 * H], BF16)
                    nc.sync.dma_start(sT[:], d_statesT.ap().rearrange("(c p) t -> p c t", p=P))
                    nc.sync.dma_start(Wq_s[:], d_Wq.ap().rearrange("(c p) e -> p c e", p=P))
                    nc.sync.dma_start(Wk_s[:], d_Wk.ap().rearrange("(c p) e -> p c e", p=P))
                    nc.sync.dma_start(Wv_s[:], d_Wv.ap().rearrange("(c p) e -> p c e", p=P))
                    nc.sync.dma_start(BD_s[:], d_BD.ap().rearrange("(c p) g -> p c g", p=P))

                    # q^T, k^T: out[e'_tile, t] ; lhsT = W[:, e'_tile], rhs = xT
                    for (Wsrc, dst, scl, bias_t) in ((Wq_s, qT, SCALE_Q, bq_t),
                                                     (Wk_s, kT, 1.0, bk_t)):
                        for m in range(EC):
                            for n in range(2):
                                ps = psA.tile([P, 512], F32, tag="ps_sc", name="ps_qk")
                                for k in range(EC):
                                    nc.tensor.matmul(
                                        ps[:], Wsrc[:, k, m * P:(m + 1) * P],
                                        sT[:, k, n * 512:(n + 1) * 512],
                                        start=(k == 0), stop=(k == EC - 1))
                                o = dst[:, m, n * 512:(n + 1) * 512]
                                if bias_t is not None:
                                    nc.vector.tensor_scalar(
                                        out=o, in0=ps[:], scalar1=scl,
                                        scalar2=bias_t[:, m:m + 1],
                                        op0=OP.mult, op1=OP.add)
                                elif scl != 1.0:
                                    nc.vector.tensor_scalar_mul(out=o, in0=ps[:],
                                                                scalar1=scl)
                                else:
                                    nc.vector.tensor_copy(out=o, in_=ps[:])

                    # v natural: out[t_tile, e'] ; lhsT = xT[:, t_tile], rhs = Wv
                    for t in range(NT):
                        for (lo, w) in ((0, 512), (512, 256)):
                            ps = psC.tile([P, 512], F32, tag="ps_proj", name="ps_v")
                            for k in range(EC):
                                nc.tensor.matmul(
                                    ps[:, 0:w], sT[:, k, t * P:(t + 1) * P],
                                    Wv_s[:, k, lo:lo + w],
                                    start=(k == 0),
                                    stop=(k == EC - 1 and not has_bv))
                            if has_bv:
                                nc.tensor.matmul(ps[:, 0:w], ones1[:],
                                                 bv_t[:, lo:lo + w],
                                                 start=False, stop=True)
                            nc.scalar.copy(out=vN[:, t, lo:lo + w], in_=ps[:, 0:w])

                    # gates: z2 = q4 @ BD  (1/SCALE_Q folded into BD on host)
                    for t in range(NT):
                        ps = psB.tile([P, 64], F32, tag="ps_ctx", name="ps_g")
                        for k in range(EC):
                            nc.tensor.matmul(ps[:, 0:2 * H],
                                             qT[:, k, t * P:(t + 1) * P],
                                             BD_s[:, k, :], start=(k == 0),
                                             stop=(k == EC - 1))
                        sig = rot.tile([P, 2 * H], F32, tag="sig", name="sig")
                        if has_bf2:
                            nc.vector.tensor_add(out=sig[:], in0=ps[:, 0:2 * H],
                                                 in1=bf2_t[:])
                            nc.scalar.activation(out=sig[:], in_=sig[:],
                                                 func=AF.Sigmoid)
                        else:
                            nc.scalar.activation(out=sig[:], in_=ps[:, 0:2 * H],
                                                 func=AF.Sigmoid)
                        ga, gb = sig[:, 0:H], sig[:, H:2 * H]
                        t1 = rot.tile([P, H], F32, tag="gt1", name="gt1")
                        nc.vector.tensor_mul(out=t1[:], in0=ga, in1=gb)
                        nc.vector.tensor_mul(out=t1[:], in0=t1[:], in1=A32_t[:])
                        nc.vector.scalar_tensor_tensor(
                            out=t1[:], in0=ga, scalar=-1.0 / 32.0, in1=t1[:],
                            op0=OP.mult, op1=OP.add)
                        nc.vector.tensor_scalar_add(out=g32[:, t, :], in0=t1[:],
                                                    scalar1=1.0 / 16.0)

                # ---------------- attention t-tile loop ----------------
                for t in range(NT):
                    t0 = t * P
                    spre_t = rot2.tile([P, E], F32, tag="spre_t", name="spre_t")
                    nc.sync.dma_start(spre_t[:], d_spre[t0:t0 + P, :])
                    for hp in range(NHP):
                        hA, hB = 2 * hp, 2 * hp + 1
                        cp = rot.tile([P, P], BF16, tag="cp", name="cp")
                        for hi, h in enumerate((hA, hB)):
                            off = hi * 64
                            pos1 = rot.tile([P, T], BF16, tag="pos1", name="pos1")
                            nc.sync.dma_start(pos1[:], d_pos[h, t0:t0 + P, :])
                            tmp = rot.tile([P, T], F32, tag="tmp", name="tmp")
                            zs = rot.tile([P, 2], F32, tag="zs", name="zs")
                            probs = rot.tile([P, T], BF16, tag="probs", name="probs")
                            for n in range(2):
                                ps = psA.tile([P, 512], F32, tag="ps_sc", name="ps_s")
                                nc.tensor.matmul(
                                    ps[:], qT[off:off + 64, hp, t0:t0 + P],
                                    kT[off:off + 64, hp, n * 512:(n + 1) * 512],
                                    tile_position=(off, 0))
                                nc.vector.scalar_tensor_tensor(
                                    out=tmp[:, n * 512:(n + 1) * 512],
                                    in0=pos1[:, n * 512:(n + 1) * 512],
                                    scalar=g32[:, t, h:h + 1], in1=ps[:],
                                    op0=OP.mult, op1=OP.add)
                                nc.scalar.activation(
                                    out=probs[:, n * 512:(n + 1) * 512],
                                    in_=tmp[:, n * 512:(n + 1) * 512],
                                    func=AF.Exp, scale=32.0,
                                    accum_out=zs[:, n:n + 1])
                            rz = rot.tile([P, 1], F32, tag="rz", name="rz")
                            nc.vector.tensor_add(out=rz[:], in0=zs[:, 0:1],
                                                 in1=zs[:, 1:2])
                            nc.vector.reciprocal(out=rz[:], in_=rz[:])
                            pT = rot.tile([P, NT, P], BF16, tag="pT", name="pT")
                            nc.scalar.dma_start(pT[:], probs[:], transpose=True)
                            psc = psB.tile([P, 64], F32, tag="ps_ctx", name="ps_c")
                            for j in range(NT):
                                nc.tensor.matmul(psc[:], pT[:, j, :],
                                                 vN[:, j, h * 64:(h + 1) * 64],
                                                 start=(j == 0), stop=(j == NT - 1))
                            nc.vector.tensor_scalar_mul(
                                out=cp[:, off:off + 64], in0=psc[:], scalar1=rz[:])
                        nc.scalar.dma_start(ctxT[:, hp, t0:t0 + P], cp[:],
                                            transpose=True)

                    # out-proj + residual + LN1 + xT
                    xp = rot2.tile([P, E], F32, tag="xp", name="xp")
                    for (lo, w) in ((0, 512), (512, 256)):
                        ps = psC.tile([P, 512], F32, tag="ps_proj", name="ps_o")
                        for k in range(EC):
                            nc.tensor.matmul(ps[:, 0:w], ctxT[:, k, t0:t0 + P],
                                             Wo_s[:, k, lo:lo + w],
                                             start=(k == 0), stop=(k == EC - 1))
                        nc.vector.tensor_add(out=xp[:, lo:lo + w],
                                             in0=spre_t[:, lo:lo + w], in1=ps[:, 0:w])
                    _layer_norm(xp[:], xN[:, t, :], ln1_t)
                    for j in range(EC):
                        nc.scalar.dma_start(xT[:, j, t0:t0 + P],
                                            xN[:, t, j * P:(j + 1) * P],
                                            transpose=True)

            # ================= FFN macro-phase =================
            with tc.tile_pool(name="ffnP", bufs=1) as ffp:
                W1_s = ffp.tile([P, EC, F], BF16)
                W2_s = ffp.tile([P, FC, E], BF16)
                gT = ffp.tile([P, FC, T], BF16)
                nc.sync.dma_start(W1_s[:], d_W1.ap().rearrange("(c p) f -> p c f", p=P))
                nc.sync.dma_start(W2_s[:], d_W2.ap().rearrange("(c p) e -> p c e", p=P))

                for half in range(2):
                    lo = half * 512
                    for f in range(FC):
                        ps = psC.tile([P, 512], F32, tag="ps_proj", name="ps_f1")
                        for k in range(EC):
                            nc.tensor.matmul(ps[:], W1_s[:, k, f * P:(f + 1) * P],
                                             xT[:, k, lo:lo + 512],
                                             start=(k == 0), stop=(k == EC - 1))
                        nc.scalar.activation(out=gT[:, f, lo:lo + 512], in_=ps[:],
                                             func=AF.Gelu, bias=b1_t[:, f:f + 1])

                for t in range(NT):
                    t0 = t * P
                    x2 = rot2.tile([P, E], F32, tag="x2", name="x2")
                    for (lo, w) in ((0, 512), (512, 256)):
                        ps = psC.tile([P, 512], F32, tag="ps_proj", name="ps_f2")
                        for f in range(FC):
                            nc.tensor.matmul(ps[:, 0:w], gT[:, f, t0:t0 + P],
                                             W2_s[:, f, lo:lo + w],
                                             start=(f == 0),
                                             stop=(f == FC - 1 and not has_b2))
                        if has_b2:
                            nc.tensor.matmul(ps[:, 0:w], ones1[:], b2_t[:, lo:lo + w],
                                             start=False, stop=True)
                        nc.vector.scalar_tensor_tensor(
                            out=x2[:, lo:lo + w], in0=xN[:, t, lo:lo + w],
                            scalar=ALPHA, in1=ps[:, 0:w], op0=OP.mult, op1=OP.add)
                    out_t = rot2.tile([P, E], F32, tag="out_t", name="out_t")
                    _layer_norm(x2[:], out_t[:], ln2_t)
                    nc.sync.dma_start(d_out[t0:t0 + P, :], out_t[:])

    nc.compile()
    return nc


def _prep(inputs):
    """Host-side prep: shard, cast, fold. Returns (flags, in_maps)."""
    bf = ml_dtypes.bfloat16
    states = np.asarray(inputs["states_in"], np.float32)
    pos = np.asarray(inputs["positional_bias"], np.float32)
    g = {k: np.asarray(v, np.float32) for k, v in inputs.items()
         if k not in ("states_in", "positional_bias")}

    has_bq = bool(np.any(g["bq"]))
    has_bk = bool(np.any(g["bk"]))
    has_bv = bool(np.any(g["bv"]))
    bf2 = g["grep_b"].reshape(2, 4).sum(-1)
    bf2_full = np.concatenate([np.full(H, bf2[0]), np.full(H, bf2[1])]).astype(np.float32)
    has_bf2 = bool(np.any(bf2))
    has_b2 = bool(np.any(g["b2"]))
    has_ln1 = not (np.all(g["ln1_g"] == 1.0) and np.all(g["ln1_b"] == 0.0))
    has_ln2 = not (np.all(g["ln2_g"] == 1.0) and np.all(g["ln2_b"] == 0.0))
    flags = (has_bq, has_bk, has_bv, has_bf2, has_b2, has_ln1, has_ln2)

    # blockdiag gate weights: ga cols 0..11, gb cols 12..23
    Wf = g["grep_W"].reshape(D, 2, 4).sum(-1) / SCALE_Q      # (64, 2), x256 folded
    BDm = np.zeros((E, 2 * H), np.float32)
    for h in range(H):
        BDm[h * D:(h + 1) * D, h] = Wf[:, 0]
        BDm[h * D:(h + 1) * D, H + h] = Wf[:, 1]

    shared = {
        "Wq": np.ascontiguousarray(g["Wq"]).astype(bf),
        "Wk": np.ascontiguousarray(g["Wk"]).astype(bf),
        "Wv": np.ascontiguousarray(g["Wv"]).astype(bf),
        "Wo": np.ascontiguousarray(g["Wo"]).astype(bf),
        "W1": np.ascontiguousarray(g["W1"]).astype(bf),
        "W2": np.ascontiguousarray(g["W2"]).astype(bf),
        "BD": BDm.astype(bf),
        "A32": (np.broadcast_to(g["grep_a"].reshape(H), (H,)) / 32.0).astype(np.float32),
        "b1": g["b1"].astype(np.float32),
    }
    if has_bq:
        shared["bqs"] = (g["bq"] * SCALE_Q).astype(np.float32)
    if has_bk:
        shared["bk"] = g["bk"].astype(np.float32)
    if has_bv:
        shared["bv"] = g["bv"].reshape(1, E).astype(bf)
    if has_bf2:
        shared["bf2"] = bf2_full
    if has_b2:
        shared["b2"] = g["b2"].reshape(1, E).astype(bf)
    if has_ln1:
        shared["ln1gb"] = np.concatenate([g["ln1_g"], g["ln1_b"]]).astype(np.float32)
    if has_ln2:
        shared["ln2gb"] = np.concatenate([g["ln2_g"], g["ln2_b"]]).astype(np.float32)

    in_maps = []
    for b in range(B):
        sb = np.ascontiguousarray(states[:, b, :])
        m = dict(shared)
        m["statesT"] = np.ascontiguousarray(sb.T).astype(bf)
        m["spre"] = (ALPHA * sb + g["bo"][None, :]).astype(np.float32)
        m["pos"] = pos[b * H:(b + 1) * H].astype(bf)
        in_maps.append(m)
    return flags, in_maps


def get_compiled(inputs):
    flags, in_maps = _prep(inputs)
    if flags not in _CACHE:
        _CACHE[flags] = _build_nc(flags)
    return _CACHE[flags], in_maps


def kernel(**inputs):
    nc, in_maps = get_compiled(inputs)
    res = run_bass_kernel_spmd(nc, in_maps, list(range(B)))
    out = np.stack([res.results[b]["out"] for b in range(B)], axis=1)
    return out.astype(np.float32), inputs["positional_bias"]
